# revision 43
# baseline (speedup 1.0000x reference)
"""Trainium2 Bass kernel for nn_AttModel (masked GNN attention).

Reference computation (per batch b of 32, N=1024, D=H=O=256):
    v = relu(x @ Wv); q = relu(x @ Wq); k = relu(x @ Wk)   (biases are zero)
    S = q @ k^T
    att = softmax(S * mask - 9e15 * (1 - mask), axis=-1)
    out = relu((att @ v) @ Wo)

Strategy: pure data parallelism over batch — 8 NeuronCores, 4 batches
each, weights replicated, no collectives.  Per batch, everything is
built around fp8 DoubleRow matmuls (0.5 cyc/row, K=256 packed per
instruction) and a transposed-S dataflow that needs no PE transposes:

  - Host packs x^T and the QKV weights as bf16 (fp8 x/W cost ~1.2e-2
    of output error, the dominant term) and the additive mask as
    float8-e4m3 (mask*31-36: a -5 softmax shift keeping exp(S-5) in
    e4m3 range, -36 masking that exp+e4m3 flushes to exact 0); Wo is
    f32r.  DRAM layouts are partition-major full-rate 2D transfers.
  - Q^T/K^T/V projections are plain bf16 matmuls; q/k/v quantize to
    e4m3 in the relu epilogues (DVE/ACT) for the fp8-DR stages.
  - S^T[m, n] = K Q^T accumulated in PSUM; the additive mask rides into
    the same accumulation group as a second fp8-DR matmul against a
    packed identity.
  - One ACT exp per m-chunk ([128,1024] PSUM -> SBUF e4m3 pm).  ACT is
    the bottleneck engine (~8.3us/batch); everything else is scheduled
    to keep it fed.
  - AV: O^T[h, n] accumulates pm-pairs straight from SBUF (fp8-DR);
    n-half 1 is deferred past the S loop to stay within 8 PSUM banks
    (st 2x2 + av 2x1 + qkv/y 2x1).
  - Softmax denominators d[n] come from Nf=1 fp8-DR matmuls (pm
    stationary, ones moving) into a spare PSUM column of the Y tile;
    Y = O^T.T @ Wo runs in f32r (fp8 there would amplify cancellation
    noise); DVE does reciprocal + (mult,max); the host unpacks the
    partition-major f32 output.
  - Emission is interleaved at m-chunk granularity: batch b+1's QKV and
    batch b-1's Y ride as fillers inside batch b's S loop so the PE,
    DVE, Pool and DMA queues all stay busy without blocking the
    exp chain.
"""

import os

import numpy as np

B, N, DIN, H, DOUT = 32, 1024, 256, 256, 256
NCORES = 8
BP = B // NCORES  # batches per core
P = 128
NSUB = N // P   # 8 m-chunks of 128
NPAIR = NSUB // 2  # 4 m-pairs (K=256 per DR matmul)

_nc_cache = {}
last_results = None  # BassKernelResults of the most recent run (for test.py)


def _build_nc(bp=BP, zero_bias=True):
    import concourse.mybir as mybir
    import concourse.tile as tile
    from concourse import bacc
    from concourse.masks import make_identity
    from contextlib import ExitStack

    f32 = mybir.dt.float32
    f32r = mybir.dt.float32r
    bf16 = mybir.dt.bfloat16
    e4 = mybir.dt.float8e4
    e5 = mybir.dt.float8e5
    AF = mybir.ActivationFunctionType
    ALU = mybir.AluOpType
    DR = mybir.MatmulPerfMode.DoubleRow

    nc = bacc.Bacc("TRN2", target_bir_lowering=False)

    # x^T packed [bp, p, c*N+n]: value x[b, n, c*128+p]
    xT_d = nc.declare_dram_parameter("xT", [bp, P, 2 * N], bf16,
                                    isOutput=False)
    # additive transposed mask [bp, p, mc*N+n]: (mask[b, n, mc*128+p]-1)*32
    mask_d = nc.declare_dram_parameter("mask", [bp, P, NSUB * N], e4,
                                       isOutput=False)
    wq_d = nc.declare_dram_parameter("Wq", [P, 2 * H], bf16, isOutput=False)
    wk_d = nc.declare_dram_parameter("Wk", [P, 2 * H], bf16, isOutput=False)
    wv_d = nc.declare_dram_parameter("Wv", [P, 2 * H], bf16, isOutput=False)
    wo_d = nc.declare_dram_parameter("Wo", [P, 2 * DOUT], f32r,
                                     isOutput=False)
    bq_d = nc.declare_dram_parameter("bq", [H, 1], f32, isOutput=False)
    bk_d = nc.declare_dram_parameter("bk", [H, 1], f32, isOutput=False)
    # y packed [bp, p, ns*DOUT+o]: value y[b, ns*128+p, o] (host unpacks)
    out_d = nc.declare_dram_parameter("out", [bp, P, NSUB * DOUT], f32,
                                      isOutput=True)

    inv = 1.0

    with tile.TileContext(nc) as tc, ExitStack() as ctx:
        const = ctx.enter_context(tc.tile_pool(name="const", bufs=1))
        sb = ctx.enter_context(tc.tile_pool(name="sb", bufs=1))
        ps = ctx.enter_context(tc.tile_pool(name="ps", bufs=1, space="PSUM"))

        st = {}

        # ---- batch-0 x first, then weights, then batch-0 mask: the sim's
        # DMA engines drain in order, so the first QT must not sit behind
        # the 1MB mask transfer ----
        def dma_x(b):
            d = st.setdefault(b, {})
            xt = sb.tile([P, 2 * N], bf16, tag="xt", bufs=3, name=f"xt{b}")
            nc.sync.dma_start(xt[:], xT_d[b])
            d["xt"] = xt

        def dma_mask(b, split=1):
            d = st.setdefault(b, {})
            mk = sb.tile([P, NSUB * N], e4, tag="mk", bufs=3, name=f"mk{b}")
            step = NSUB * N // split
            for s in range(split):
                nc.sync.dma_start(mk[:, s * step:(s + 1) * step],
                                  mask_d[b, :, s * step:(s + 1) * step])
            d["mk"] = mk

        def dma_in(b):
            dma_x(b)
            dma_mask(b)

        # batch-0 path: x, first mask quarter and wq/wk land first so the
        # first S^T chunk starts ~4us in; the serial DMA engine otherwise
        # parks the whole 1MB mask in front of the weights
        dma_x(0)
        mk0 = sb.tile([P, NSUB * N], e4, tag="mk", bufs=3, name="mk0")
        st.setdefault(0, {})["mk"] = mk0
        nc.sync.dma_start(mk0[:, :2 * N], mask_d[0, :, :2 * N])
        wq_sb = const.tile([P, 2 * H], bf16, tag="wq", name="wq_sb")
        nc.sync.dma_start(wq_sb[:], wq_d[:])
        wk_sb = const.tile([P, 2 * H], bf16, tag="wk", name="wk_sb")
        nc.sync.dma_start(wk_sb[:], wk_d[:])
        nc.sync.dma_start(mk0[:, 2 * N:4 * N], mask_d[0, :, 2 * N:4 * N])
        wv_sb = const.tile([P, 2 * H], bf16, tag="wv", name="wv_sb")
        nc.sync.dma_start(wv_sb[:], wv_d[:])
        if not zero_bias:
            bq_sb = const.tile([P, 2], f32, tag="bq", name="bq_sb")
            nc.gpsimd.dma_start(
                bq_sb[:].rearrange("p (c o) -> p c o", c=2),
                bq_d.rearrange("(c p) o -> p c o", c=2))
            bk_sb = const.tile([P, 2], f32, tag="bk", name="bk_sb")
            nc.gpsimd.dma_start(
                bk_sb[:].rearrange("p (c o) -> p c o", c=2),
                bk_d.rearrange("(c p) o -> p c o", c=2))

        # identity packs for the mask-add matmul: idp[nh] has I at k-tile nh
        idp = []
        for nh in range(2):
            t = const.tile([P, 2 * P], e4, tag=f"idp{nh}", name=f"idp{nh}")
            nc.gpsimd.memset(t[:], 0.0)
            make_identity(nc, t[:, nh * P:(nh + 1) * P], nomemset=True)
            idp.append(t)
        ones2 = const.tile([P, 2], e4, tag="ones2", name="ones2")
        nc.gpsimd.memset(ones2[:], 1.0)

        wo_sb = const.tile([P, 2 * DOUT], f32r, tag="wo", name="wo_sb")
        nc.sync.dma_start(wo_sb[:], wo_d[:])
        nc.sync.dma_start(mk0[:, 4 * N:], mask_d[0, :, 4 * N:])

        def qkv0_pieces():
            """Batch-0 QKV with QT/V on the idle st-ring (4 banks) and KT on
            the ps-ring, so the startup is not paced by a single 2-deep
            ring; steady-state batches overlap a full S phase instead."""
            d = st[0]
            alloc_qk(0)
            d["v"] = []
            # nh=0 halves first: the first S^T chunk needs only qtA/ktA
            for nh in range(2):
                for hc in range(2):
                    pq = ps.tile([P, N], f32, tag="st", bufs=2,
                                 name=f"pq0_{hc}_{nh}")
                    for dc in range(2):
                        nc.tensor.matmul(
                            pq[:, 0:512],
                            wq_sb[:, dc * H + hc * P:dc * H + (hc + 1) * P],
                            d["xt"][:, dc * N + nh * 512:
                                    dc * N + (nh + 1) * 512],
                            start=(dc == 0), stop=(dc == 1))
                    nc.vector.tensor_scalar(
                        out=d["qtn"][nh][:, hc * 512:(hc + 1) * 512],
                        in0=pq[:, 0:512], scalar1=inv,
                        scalar2=0.0, op0=ALU.mult, op1=ALU.max)
                    pk = ps.tile([P, 512], f32, tag="ps", bufs=2,
                                 name=f"pk0_{hc}_{nh}")
                    for dc in range(2):
                        nc.tensor.matmul(
                            pk[:],
                            wk_sb[:, dc * H + hc * P:dc * H + (hc + 1) * P],
                            d["xt"][:, dc * N + nh * 512:
                                    dc * N + (nh + 1) * 512],
                            start=(dc == 0), stop=(dc == 1))
                    nc.scalar.activation(
                        d["ktn"][nh][:, hc * 512:(hc + 1) * 512], pk[:],
                        AF.Relu, scale=inv)
            def emit_v0(jj):
                pv = ps.tile([P, 512], f32, tag="ps", bufs=2,
                             name=f"pv0_{jj}")
                for c2 in range(2):
                    mc = 2 * jj + c2
                    for dc in range(2):
                        nc.tensor.matmul(
                            pv[:, c2 * H:(c2 + 1) * H],
                            d["xt"][:, dc * N + mc * P:dc * N + (mc + 1) * P],
                            wv_sb[:, dc * H:(dc + 1) * H],
                            start=(dc == 0), stop=(dc == 1))
                v = sb.tile([P, 2 * H], e4, tag="v", bufs=2 * NPAIR,
                            name=f"v0_{jj}")
                if jj % 2:
                    nc.vector.tensor_scalar(
                        out=v[:], in0=pv[:], scalar1=inv, scalar2=0.0,
                        op0=ALU.mult, op1=ALU.max)
                else:
                    nc.scalar.activation(v[:], pv[:], AF.Relu, scale=inv)
                d["v"].append(v)

            # V is first needed at mc2 of s(0): ride the fillers instead of
            # blocking the first S^T chunks in the in-order PE queue
            return [lambda jj=jj: emit_v0(jj) for jj in range(NPAIR)]

        def alloc_qk(b):
            # q and k live as per-n-half / per-m-half tiles ([p, (hc n)]) so
            # a consumer's (coarse, tile-granular) dependency covers only
            # the half it actually reads
            d = st.setdefault(b, {})
            d["qtn"] = [sb.tile([P, N], e4, tag=f"qt{nh}", bufs=2,
                                name=f"qt{b}_{nh}") for nh in range(2)]
            d["ktn"] = [sb.tile([P, N], e4, tag=f"kt{nh}", bufs=2,
                                name=f"kt{b}_{nh}") for nh in range(2)]

        def qkv_pieces(b):
            """Returns emission closures: Q^T/K^T/V projections for batch b.
            State is resolved lazily so the dma_in(b) filler can run first."""

            def prelude():
                alloc_qk(b)
                st[b]["v"] = []

            def emit_qk(use_q, hc, nh, on_dve):
                d = st[b]
                w_sb = wq_sb if use_q else wk_sb
                dst = (d["qtn"] if use_q else d["ktn"])[nh]
                pq = ps.tile([P, 512], f32, tag="ps", bufs=2,
                             name=f"pqk{b}_{hc}_{nh}")
                for dc in range(2):
                    nc.tensor.matmul(
                        pq[:],
                        w_sb[:, dc * H + hc * P:dc * H + (hc + 1) * P],
                        d["xt"][:, dc * N + nh * 512:dc * N + (nh + 1) * 512],
                        start=(dc == 0), stop=(dc == 1))
                qsl = slice(hc * 512, (hc + 1) * 512)
                if not zero_bias:
                    bias = (bq_sb if use_q else bk_sb)[:, hc:hc + 1]
                    nc.scalar.activation(dst[:, qsl], pq[:], AF.Relu,
                                         bias=bias, scale=inv)
                elif on_dve:
                    nc.vector.tensor_scalar(
                        out=dst[:, qsl], in0=pq[:], scalar1=inv,
                        scalar2=0.0, op0=ALU.mult, op1=ALU.max)
                else:
                    nc.scalar.activation(dst[:, qsl], pq[:], AF.Relu,
                                         scale=inv)

            def emit_v(j):
                d = st[b]
                pv = ps.tile([P, 512], f32, tag="ps", bufs=2,
                             name=f"pv{b}_{j}")
                for c2 in range(2):
                    mc = 2 * j + c2
                    for dc in range(2):
                        nc.tensor.matmul(
                            pv[:, c2 * H:(c2 + 1) * H],
                            d["xt"][:, dc * N + mc * P:dc * N + (mc + 1) * P],
                            wv_sb[:, dc * H:(dc + 1) * H],
                            start=(dc == 0), stop=(dc == 1))
                v = sb.tile([P, 2 * H], e4, tag="v", bufs=2 * NPAIR,
                            name=f"v{b}_{j}")
                nc.vector.tensor_scalar(
                    out=v[:], in0=pv[:], scalar1=inv, scalar2=0.0,
                    op0=ALU.mult, op1=ALU.max)
                d["v"].append(v)

            pieces = [prelude]
            for hc in range(2):
                for nh in range(2):
                    # alternate DVE (q) / Pool (k) so neither queue bursts
                    pieces.append(
                        lambda hc=hc, nh=nh: emit_qk(True, hc, nh, True))
                    # kt epilogues split ACT (nh0) / DVE (nh1) for balance
                    pieces.append(
                        lambda hc=hc, nh=nh: emit_qk(False, hc, nh, nh == 1))
            for j in range(NPAIR):
                # one v epilogue per batch goes to DVE to balance Pool
                pieces.append(lambda j=j: emit_v(j))
            return pieces

        def s_phase(b, fillers=()):
            """S^T + mask (PE) -> exp (ACT) -> AV n-half 0 (PE), with
            filler closures from other batches drained between chunks."""
            d = st[b]
            qt3 = [t[:].rearrange("p (c n) -> p c n", c=2) for t in d["qtn"]]
            kt3 = [t[:].rearrange("p (c n) -> p c n", c=2) for t in d["ktn"]]
            mk = d["mk"]
            fillers = list(fillers)
            fpc = (len(fillers) + NSUB - 1) // NSUB if fillers else 0
            pms = []
            d["pm"] = pms  # filled as the loop runs; read by emit_av0/trav_a
            for mc in range(NSUB):
                stp = ps.tile([P, N], f32, tag="st", bufs=2,
                              name=f"st{b}_{mc}")
                mk3 = mk[:, mc * N:(mc + 1) * N].rearrange(
                    "p (c n) -> p c n", c=2)
                for nh in range(2):
                    nsl = slice(nh * 512, (nh + 1) * 512)
                    nc.tensor.matmul(
                        stp[:, nsl],
                        kt3[mc // 4][:, :, (mc % 4) * P:(mc % 4 + 1) * P],
                        qt3[nh][:], start=True, stop=False, perf_mode=DR)
                    nc.tensor.matmul(
                        stp[:, nsl],
                        idp[nh][:].rearrange("p (c m) -> p c m", c=2),
                        mk3[:], start=False, stop=True, perf_mode=DR)
                if mc % 2 == 0:
                    pm = sb.tile([P, 2 * N], e4, tag="pm", bufs=3 * NPAIR,
                                 name=f"pm{b}_{mc // 2}")
                    pms.append(pm)
                nc.scalar.activation(
                    pms[-1][:, (mc % 2) * N:(mc % 2 + 1) * N], stp[:], AF.Exp)
                # AV0 for pair j is emitted two chunks late (at mc=2j+3) so
                # its wait on exp(2j+1) never sits ahead of the next S^T in
                # the in-order PE queue; the last pair lands in trav_a.
                if mc % 2 == 1 and mc >= 3:
                    j = mc // 2 - 1
                    if j == 0:
                        # lazy alloc keeps the "av" ring ordered with the
                        # previous batch's deferred av1 tiles
                        d["av0"] = [ps.tile([P, 512], f32, tag="av", bufs=2,
                                            name=f"av0_{b}_{hc}")
                                    for hc in range(2)]
                    emit_av0(b, j)
                for _ in range(fpc):
                    if fillers:
                        fillers.pop(0)()
            while fillers:
                fillers.pop(0)()

        def emit_av0(b, j):
            d = st[b]
            pm3 = d["pm"][j][:].rearrange("p (c n) -> p c n", c=2)
            v3 = d["v"][j][:].rearrange("p (c h) -> p c h", c=2)
            for hc in range(2):
                nc.tensor.matmul(
                    d["av0"][hc][:], v3[:, :, hc * P:(hc + 1) * P],
                    pm3[:, :, 0:512], start=(j == 0),
                    stop=(j == NPAIR - 1), perf_mode=DR)

        def trav_y_pieces(b):
            """Closures for the post-S work of batch b: deferred AV n-half 1,
            O^T copies, per-n-chunk d+Y+epilogue, chunked output DMAs.
            Run as fillers inside s(b+1) so nothing blocks its exp chain."""
            ones3 = ones2[:].rearrange("p (c o) -> p c o", c=2)

            def trav_a():
                d = st[b]
                emit_av0(b, NPAIR - 1)  # deferred last pair
                d["ot"] = [sb.tile([P, N], f32r, tag="ot", bufs=4,
                                   name=f"ot{b}_{hc}") for hc in range(2)]
                if b == bp - 1:
                    # tail: the st-ring banks are free once the last exp has
                    # read them — av1 there skips the av-ring rotation
                    av1 = [ps.tile([P, N], f32, tag="st", bufs=2,
                                   name=f"av1_{b}_{hc}")[:, 0:512]
                           for hc in range(2)]
                else:
                    av1 = [ps.tile([P, 512], f32, tag="av", bufs=2,
                                   name=f"av1_{b}_{hc}")[:]
                           for hc in range(2)]
                d["av1"] = av1
                nc.scalar.copy(d["ot"][0][:, 0:512], d["av0"][0][:])
                nc.vector.tensor_copy(d["ot"][1][:, 0:512], d["av0"][1][:])
                for j in range(NPAIR):
                    pm3 = d["pm"][j][:].rearrange("p (c n) -> p c n", c=2)
                    v3 = d["v"][j][:].rearrange("p (c h) -> p c h", c=2)
                    for hc in range(2):
                        nc.tensor.matmul(
                            av1[hc], v3[:, :, hc * P:(hc + 1) * P],
                            pm3[:, :, 512:1024], start=(j == 0),
                            stop=(j == NPAIR - 1), perf_mode=DR)

            def trav_b():
                d = st[b]
                nc.scalar.copy(d["ot"][0][:, 512:1024], d["av1"][0])
                nc.vector.tensor_copy(d["ot"][1][:, 512:1024], d["av1"][1])

            def y_pre():
                st[b]["ybig"] = sb.tile([P, NSUB * DOUT], f32, tag="y",
                                        bufs=2, name=f"y{b}")

            def emit_y(ns):
                d = st[b]
                nsl = slice(ns * P, (ns + 1) * P)
                yp = ps.tile([P, 512], f32, tag="ps", bufs=2,
                             name=f"yp{b}_{ns}")
                for j in range(NPAIR):
                    pm3 = d["pm"][j][:].rearrange("p (c n) -> p c n", c=2)
                    nc.tensor.matmul(
                        yp[:, DOUT:DOUT + 1], pm3[:, :, nsl], ones3[:],
                        start=(j == 0), stop=(j == NPAIR - 1), perf_mode=DR)
                for hc in range(2):
                    nc.tensor.matmul(
                        yp[:, 0:DOUT], d["ot"][hc][:, nsl],
                        wo_sb[:, hc * DOUT:(hc + 1) * DOUT],
                        start=(hc == 0), stop=(hc == 1))
                iv = sb.tile([P, 1], f32, tag="iv", bufs=4,
                             name=f"iv{b}_{ns}")
                nc.vector.reciprocal(iv[:], yp[:, DOUT:DOUT + 1])
                if ns % 2 and b == bp - 1:
                    nc.scalar.activation(
                        ybig_of(b)[:, ns * DOUT:(ns + 1) * DOUT],
                        yp[:, 0:DOUT], AF.Relu, scale=iv[:])
                else:
                    nc.vector.tensor_scalar(
                        out=ybig_of(b)[:, ns * DOUT:(ns + 1) * DOUT],
                        in0=yp[:, 0:DOUT],
                        scalar1=iv[:], scalar2=0.0, op0=ALU.mult, op1=ALU.max)

            def ybig_of(b):
                return st[b]["ybig"]

            def emit_out(q):
                # quarter-batch output DMA right after its data is ready so
                # the SP queue is never held on a long semaphore wait
                csl = slice(q * 2 * DOUT, (q + 1) * 2 * DOUT)
                nc.sync.dma_start(out_d[b, :, csl], ybig_of(b)[:, csl])
                if q == 3:
                    del st[b]

            pieces = [trav_a, trav_b, y_pre]
            for ns in range(NSUB):
                pieces.append(lambda ns=ns: emit_y(ns))
                if ns % 2 == 1:
                    pieces.append(lambda q=ns // 2: emit_out(q))
            return pieces

        # ---- interleaved emission ----
        # s(b) drains fillers between m-chunks: the previous batch's
        # trav/Y/output pieces merged round-robin with batch b+1's input
        # DMAs and QKV so the epilogue engines never burst.
        v0_pieces = qkv0_pieces()
        prev = []
        for b in range(bp):
            nxt = list(v0_pieces) if b == 0 else []
            v0_pieces = []
            if b + 1 < bp:
                nxt.append(lambda bb=b + 1: dma_in(bb))
                nxt.extend(qkv_pieces(b + 1))
            a, c = list(prev), list(nxt)
            fillers = []
            while a or c:
                if a:
                    fillers.append(a.pop(0))
                for _ in range(2):
                    if c:
                        fillers.append(c.pop(0))
            s_phase(b, fillers)
            prev = trav_y_pieces(b)
        for f in prev:
            f()

    nc.compile()
    return nc


def _get_nc(bp=BP, zero_bias=True):
    key = (bp, zero_bias)
    if key not in _nc_cache:
        _nc_cache[key] = _build_nc(bp, zero_bias)
    return _nc_cache[key]


def _pack_inputs(x, mask, Wv, Wk, Wq, Wo, bq, bk):
    import ml_dtypes

    e4 = ml_dtypes.float8_e4m3
    bf = ml_dtypes.bfloat16
    x = np.asarray(x, np.float32)
    b = x.shape[0]
    # x^T packed [b, p, c*N+n]
    xT = x.transpose(0, 2, 1).reshape(b, 2, P, N).transpose(0, 2, 1, 3)
    xT = np.ascontiguousarray(xT.reshape(b, P, 2 * N)).astype(bf)
    # (mask^T - 1) * 32 packed [b, p, mc*N+n]
    # additive mask {unmasked: -5, masked: -36}: a -5 softmax shift
    # keeps exp(S-5) inside float8-e4m3 range; -36 flushes to exact 0
    mk = np.asarray(mask, np.float32).transpose(0, 2, 1) * 31.0 - 36.0
    mk = mk.reshape(b, NSUB, P, N).transpose(0, 2, 1, 3)
    mk = np.ascontiguousarray(mk.reshape(b, P, NSUB * N)).astype(e4)

    def packw(w, dt, scale):
        w = np.asarray(w, np.float32) * scale
        return np.ascontiguousarray(
            w.reshape(2, P, -1).transpose(1, 0, 2).reshape(P, -1)).astype(dt)

    return {
        "xT": xT, "mask": mk,
        "Wq": packw(Wq, bf, 1.0),
        "Wk": packw(Wk, bf, 1.0),
        "Wv": packw(Wv, bf, 1.0),
        "Wo": packw(Wo, np.float32, 1.0),
        "bq": np.asarray(bq, np.float32).reshape(H, 1).copy(),
        "bk": np.asarray(bk, np.float32).reshape(H, 1).copy(),
    }


def kernel(x, mask, Wv, bv, Wk, bk, Wq, bq, Wo, bo):
    global last_results
    from concourse.bass_utils import run_bass_kernel_spmd

    if np.any(np.asarray(bv, np.float32)) or np.any(np.asarray(bo, np.float32)):
        raise NotImplementedError("nonzero bv/bo not supported")
    zero_bias = not (np.any(np.asarray(bq, np.float32))
                     or np.any(np.asarray(bk, np.float32)))

    w = _pack_inputs(x, mask, Wv, Wk, Wq, Wo, bq, bk)
    nc = _get_nc(BP, zero_bias)
    in_maps = []
    for c in range(NCORES):
        sl = slice(c * BP, (c + 1) * BP)
        m = {"xT": np.ascontiguousarray(w["xT"][sl]),
             "mask": np.ascontiguousarray(w["mask"][sl])}
        for k in ("Wq", "Wk", "Wv", "Wo", "bq", "bk"):
            m[k] = w[k]
        in_maps.append(m)

    trace = bool(int(os.environ.get("BASS_KERNEL_TRACE", "0")))
    try:
        res = run_bass_kernel_spmd(
            nc, in_maps, core_ids=list(range(NCORES)), trace=trace
        )
    except Exception:
        if not trace:
            raise
        res = run_bass_kernel_spmd(nc, in_maps, core_ids=list(range(NCORES)))
    last_results = res
    # out comes back packed [bp, p, ns*DOUT+o] bf16 -> [B, N, DOUT] f32
    outs = []
    for r in res.results:
        y = np.asarray(r["out"], np.float32).reshape(BP, P, NSUB, DOUT)
        outs.append(y.transpose(0, 2, 1, 3).reshape(BP, N, DOUT))
    return np.ascontiguousarray(np.concatenate(outs, axis=0))


if __name__ == "__main__":
    nc = _get_nc(1)
    print("built ok:", nc)


# revision 46
# speedup vs baseline: 1.0062x; 1.0062x over previous
"""Trainium2 Bass kernel for nn_AttModel (masked GNN attention).

Reference computation (per batch b of 32, N=1024, D=H=O=256):
    v = relu(x @ Wv); q = relu(x @ Wq); k = relu(x @ Wk)   (biases are zero)
    S = q @ k^T
    att = softmax(S * mask - 9e15 * (1 - mask), axis=-1)
    out = relu((att @ v) @ Wo)

Strategy: pure data parallelism over batch — 8 NeuronCores, 4 batches
each, weights replicated, no collectives.  Per batch, everything is
built around fp8 DoubleRow matmuls (0.5 cyc/row, K=256 packed per
instruction) and a transposed-S dataflow that needs no PE transposes:

  - Host packs x^T and the QKV weights as bf16 (fp8 x/W cost ~1.2e-2
    of output error, the dominant term) and the additive mask as
    float8-e4m3 (mask*31-36: a -5 softmax shift keeping exp(S-5) in
    e4m3 range, -36 masking that exp+e4m3 flushes to exact 0); Wo is
    f32r.  DRAM layouts are partition-major full-rate 2D transfers.
  - Q^T/K^T/V projections are plain bf16 matmuls; q/k/v quantize to
    e4m3 in the relu epilogues (DVE/ACT) for the fp8-DR stages.
  - S^T[m, n] = K Q^T accumulated in PSUM; the additive mask rides into
    the same accumulation group as a second fp8-DR matmul against a
    packed identity.
  - One ACT exp per m-chunk ([128,1024] PSUM -> SBUF e4m3 pm).  ACT is
    the bottleneck engine (~8.3us/batch); everything else is scheduled
    to keep it fed.
  - AV: O^T[h, n] accumulates pm-pairs straight from SBUF (fp8-DR);
    n-half 1 is deferred past the S loop to stay within 8 PSUM banks
    (st 2x2 + av 2x1 + qkv/y 2x1).
  - Softmax denominators d[n] come from Nf=1 fp8-DR matmuls (pm
    stationary, ones moving) into a spare PSUM column of the Y tile;
    Y = O^T.T @ Wo runs in f32r (fp8 there would amplify cancellation
    noise); DVE does reciprocal + (mult,max); the host unpacks the
    partition-major f32 output.
  - Emission is interleaved at m-chunk granularity: batch b+1's QKV and
    batch b-1's Y ride as fillers inside batch b's S loop so the PE,
    DVE, Pool and DMA queues all stay busy without blocking the
    exp chain.
"""

import os

import numpy as np

B, N, DIN, H, DOUT = 32, 1024, 256, 256, 256
NCORES = 8
BP = B // NCORES  # batches per core
P = 128
NSUB = N // P   # 8 m-chunks of 128
NPAIR = NSUB // 2  # 4 m-pairs (K=256 per DR matmul)

_nc_cache = {}
last_results = None  # BassKernelResults of the most recent run (for test.py)


def _build_nc(bp=BP, zero_bias=True):
    import concourse.mybir as mybir
    import concourse.tile as tile
    from concourse import bacc
    from concourse.masks import make_identity
    from contextlib import ExitStack

    f32 = mybir.dt.float32
    f32r = mybir.dt.float32r
    bf16 = mybir.dt.bfloat16
    e4 = mybir.dt.float8e4
    e5 = mybir.dt.float8e5
    AF = mybir.ActivationFunctionType
    ALU = mybir.AluOpType
    DR = mybir.MatmulPerfMode.DoubleRow

    nc = bacc.Bacc("TRN2", target_bir_lowering=False)

    # x^T packed [bp, p, c*N+n]: value x[b, n, c*128+p]
    xT_d = nc.declare_dram_parameter("xT", [bp, P, 2 * N], bf16,
                                    isOutput=False)
    xT8_d = nc.declare_dram_parameter("xT8", [bp, P, 2 * N], e4,
                                      isOutput=False)
    # additive transposed mask [bp, p, mc*N+n]: (mask[b, n, mc*128+p]-1)*32
    mask_d = nc.declare_dram_parameter("mask", [bp, P, NSUB * N], e4,
                                       isOutput=False)
    wq_d = nc.declare_dram_parameter("Wq", [P, 2 * H], bf16, isOutput=False)
    wk_d = nc.declare_dram_parameter("Wk", [P, 2 * H], bf16, isOutput=False)
    wv_d = nc.declare_dram_parameter("Wv", [P, 2 * H], e4, isOutput=False)
    wo_d = nc.declare_dram_parameter("Wo", [P, 2 * DOUT], f32r,
                                     isOutput=False)
    bq_d = nc.declare_dram_parameter("bq", [H, 1], f32, isOutput=False)
    bk_d = nc.declare_dram_parameter("bk", [H, 1], f32, isOutput=False)
    # y packed [bp, p, ns*DOUT+o]: value y[b, ns*128+p, o] (host unpacks)
    out_d = nc.declare_dram_parameter("out", [bp, P, NSUB * DOUT], f32,
                                      isOutput=True)

    inv = 1.0
    inv32 = 1.0 / 32.0

    with tile.TileContext(nc) as tc, ExitStack() as ctx:
        const = ctx.enter_context(tc.tile_pool(name="const", bufs=1))
        sb = ctx.enter_context(tc.tile_pool(name="sb", bufs=1))
        ps = ctx.enter_context(tc.tile_pool(name="ps", bufs=1, space="PSUM"))

        st = {}

        # ---- batch-0 x first, then weights, then batch-0 mask: the sim's
        # DMA engines drain in order, so the first QT must not sit behind
        # the 1MB mask transfer ----
        def dma_x(b):
            d = st.setdefault(b, {})
            xt = sb.tile([P, 2 * N], bf16, tag="xt", bufs=3, name=f"xt{b}")
            nc.sync.dma_start(xt[:], xT_d[b])
            xt8 = sb.tile([P, 2 * N], e4, tag="xt8", bufs=3, name=f"xt8{b}")
            nc.sync.dma_start(xt8[:], xT8_d[b])
            d["xt"], d["xt8"] = xt, xt8

        def dma_mask(b, split=1):
            d = st.setdefault(b, {})
            mk = sb.tile([P, NSUB * N], e4, tag="mk", bufs=3, name=f"mk{b}")
            step = NSUB * N // split
            for s in range(split):
                nc.sync.dma_start(mk[:, s * step:(s + 1) * step],
                                  mask_d[b, :, s * step:(s + 1) * step])
            d["mk"] = mk

        def dma_in(b):
            dma_x(b)
            dma_mask(b)

        # batch-0 path: x, first mask quarter and wq/wk land first so the
        # first S^T chunk starts ~4us in; the serial DMA engine otherwise
        # parks the whole 1MB mask in front of the weights
        dma_x(0)
        mk0 = sb.tile([P, NSUB * N], e4, tag="mk", bufs=3, name="mk0")
        st.setdefault(0, {})["mk"] = mk0
        nc.sync.dma_start(mk0[:, :2 * N], mask_d[0, :, :2 * N])
        wq_sb = const.tile([P, 2 * H], bf16, tag="wq", name="wq_sb")
        nc.sync.dma_start(wq_sb[:], wq_d[:])
        wk_sb = const.tile([P, 2 * H], bf16, tag="wk", name="wk_sb")
        nc.sync.dma_start(wk_sb[:], wk_d[:])
        nc.sync.dma_start(mk0[:, 2 * N:4 * N], mask_d[0, :, 2 * N:4 * N])
        wv_sb = const.tile([P, 2 * H], e4, tag="wv", name="wv_sb")
        nc.sync.dma_start(wv_sb[:], wv_d[:])
        if not zero_bias:
            bq_sb = const.tile([P, 2], f32, tag="bq", name="bq_sb")
            nc.gpsimd.dma_start(
                bq_sb[:].rearrange("p (c o) -> p c o", c=2),
                bq_d.rearrange("(c p) o -> p c o", c=2))
            bk_sb = const.tile([P, 2], f32, tag="bk", name="bk_sb")
            nc.gpsimd.dma_start(
                bk_sb[:].rearrange("p (c o) -> p c o", c=2),
                bk_d.rearrange("(c p) o -> p c o", c=2))

        # identity packs for the mask-add matmul: idp[nh] has I at k-tile nh
        idp = []
        for nh in range(2):
            t = const.tile([P, 2 * P], e4, tag=f"idp{nh}", name=f"idp{nh}")
            nc.gpsimd.memset(t[:], 0.0)
            make_identity(nc, t[:, nh * P:(nh + 1) * P], nomemset=True)
            idp.append(t)
        ones2 = const.tile([P, 2], e4, tag="ones2", name="ones2")
        nc.gpsimd.memset(ones2[:], 1.0)

        wo_sb = const.tile([P, 2 * DOUT], f32r, tag="wo", name="wo_sb")
        nc.sync.dma_start(wo_sb[:], wo_d[:])
        nc.sync.dma_start(mk0[:, 4 * N:], mask_d[0, :, 4 * N:])

        def qkv0_pieces():
            """Batch-0 QKV with QT/V on the idle st-ring (4 banks) and KT on
            the ps-ring, so the startup is not paced by a single 2-deep
            ring; steady-state batches overlap a full S phase instead."""
            d = st[0]
            alloc_qk(0)
            d["v"] = []
            # nh=0 halves first: the first S^T chunk needs only qtA/ktA
            for nh in range(2):
                for hc in range(2):
                    pq = ps.tile([P, N], f32, tag="st", bufs=2,
                                 name=f"pq0_{hc}_{nh}")
                    for dc in range(2):
                        nc.tensor.matmul(
                            pq[:, 0:512],
                            wq_sb[:, dc * H + hc * P:dc * H + (hc + 1) * P],
                            d["xt"][:, dc * N + nh * 512:
                                    dc * N + (nh + 1) * 512],
                            start=(dc == 0), stop=(dc == 1))
                    nc.vector.tensor_scalar(
                        out=d["qtn"][nh][:, hc * 512:(hc + 1) * 512],
                        in0=pq[:, 0:512], scalar1=inv,
                        scalar2=0.0, op0=ALU.mult, op1=ALU.max)
                    pk = ps.tile([P, 512], f32, tag="ps", bufs=2,
                                 name=f"pk0_{hc}_{nh}")
                    for dc in range(2):
                        nc.tensor.matmul(
                            pk[:],
                            wk_sb[:, dc * H + hc * P:dc * H + (hc + 1) * P],
                            d["xt"][:, dc * N + nh * 512:
                                    dc * N + (nh + 1) * 512],
                            start=(dc == 0), stop=(dc == 1))
                    if hc == 0:
                        nc.scalar.activation(
                            d["ktn"][nh][:, hc * 512:(hc + 1) * 512], pk[:],
                            AF.Relu, scale=inv)
                    else:
                        nc.vector.tensor_scalar(
                            out=d["ktn"][nh][:, hc * 512:(hc + 1) * 512],
                            in0=pk[:], scalar1=inv, scalar2=0.0,
                            op0=ALU.mult, op1=ALU.max)
            def emit_v0(jj):
                pv = ps.tile([P, 512], f32, tag="ps", bufs=2,
                             name=f"pv0_{jj}")
                xt83 = d["xt8"][:].rearrange("p (c n) -> p c n", c=2)
                wv3 = wv_sb[:].rearrange("p (c h) -> p c h", c=2)
                for c2 in range(2):
                    mc = 2 * jj + c2
                    nc.tensor.matmul(
                        pv[:, c2 * H:(c2 + 1) * H],
                        xt83[:, :, mc * P:(mc + 1) * P], wv3[:],
                        start=True, stop=True, perf_mode=DR)
                v = sb.tile([P, 2 * H], e4, tag="v", bufs=2 * NPAIR,
                            name=f"v0_{jj}")
                if jj % 2:
                    nc.vector.tensor_scalar(
                        out=v[:], in0=pv[:], scalar1=inv32, scalar2=0.0,
                        op0=ALU.mult, op1=ALU.max)
                else:
                    nc.scalar.activation(v[:], pv[:], AF.Relu, scale=inv32)
                d["v"].append(v)

            # V is first needed at mc2 of s(0): ride the fillers instead of
            # blocking the first S^T chunks in the in-order PE queue
            return [lambda jj=jj: emit_v0(jj) for jj in range(NPAIR)]

        def alloc_qk(b):
            # q and k live as per-n-half / per-m-half tiles ([p, (hc n)]) so
            # a consumer's (coarse, tile-granular) dependency covers only
            # the half it actually reads
            d = st.setdefault(b, {})
            d["qtn"] = [sb.tile([P, N], e4, tag=f"qt{nh}", bufs=2,
                                name=f"qt{b}_{nh}") for nh in range(2)]
            d["ktn"] = [sb.tile([P, N], e4, tag=f"kt{nh}", bufs=2,
                                name=f"kt{b}_{nh}") for nh in range(2)]

        def qkv_pieces(b):
            """Returns emission closures: Q^T/K^T/V projections for batch b.
            State is resolved lazily so the dma_in(b) filler can run first."""

            def prelude():
                alloc_qk(b)
                st[b]["v"] = []

            def emit_qk(use_q, hc, nh, on_dve):
                d = st[b]
                w_sb = wq_sb if use_q else wk_sb
                dst = (d["qtn"] if use_q else d["ktn"])[nh]
                pq = ps.tile([P, 512], f32, tag="ps", bufs=2,
                             name=f"pqk{b}_{hc}_{nh}")
                for dc in range(2):
                    nc.tensor.matmul(
                        pq[:],
                        w_sb[:, dc * H + hc * P:dc * H + (hc + 1) * P],
                        d["xt"][:, dc * N + nh * 512:dc * N + (nh + 1) * 512],
                        start=(dc == 0), stop=(dc == 1))
                qsl = slice(hc * 512, (hc + 1) * 512)
                if not zero_bias:
                    bias = (bq_sb if use_q else bk_sb)[:, hc:hc + 1]
                    nc.scalar.activation(dst[:, qsl], pq[:], AF.Relu,
                                         bias=bias, scale=inv)
                elif on_dve:
                    nc.vector.tensor_scalar(
                        out=dst[:, qsl], in0=pq[:], scalar1=inv,
                        scalar2=0.0, op0=ALU.mult, op1=ALU.max)
                else:
                    nc.scalar.activation(dst[:, qsl], pq[:], AF.Relu,
                                         scale=inv)

            def emit_v(j):
                d = st[b]
                pv = ps.tile([P, 512], f32, tag="ps", bufs=2,
                             name=f"pv{b}_{j}")
                xt83 = d["xt8"][:].rearrange("p (c n) -> p c n", c=2)
                wv3 = wv_sb[:].rearrange("p (c h) -> p c h", c=2)
                for c2 in range(2):
                    mc = 2 * j + c2
                    nc.tensor.matmul(
                        pv[:, c2 * H:(c2 + 1) * H],
                        xt83[:, :, mc * P:(mc + 1) * P], wv3[:],
                        start=True, stop=True, perf_mode=DR)
                v = sb.tile([P, 2 * H], e4, tag="v", bufs=2 * NPAIR,
                            name=f"v{b}_{j}")
                nc.vector.tensor_scalar(
                    out=v[:], in0=pv[:], scalar1=inv32, scalar2=0.0,
                    op0=ALU.mult, op1=ALU.max)
                d["v"].append(v)

            pieces = [prelude]
            for hc in range(2):
                for nh in range(2):
                    # alternate DVE (q) / Pool (k) so neither queue bursts
                    pieces.append(
                        lambda hc=hc, nh=nh: emit_qk(True, hc, nh, True))
                    # kt epilogues split ACT (nh0) / DVE (nh1) for balance
                    pieces.append(
                        lambda hc=hc, nh=nh: emit_qk(False, hc, nh, nh == 1))
            for j in range(NPAIR):
                # one v epilogue per batch goes to DVE to balance Pool
                pieces.append(lambda j=j: emit_v(j))
            return pieces

        def s_phase(b, fillers=()):
            """S^T + mask (PE) -> exp (ACT) -> AV n-half 0 (PE), with
            filler closures from other batches drained between chunks."""
            d = st[b]
            qt3 = [t[:].rearrange("p (c n) -> p c n", c=2) for t in d["qtn"]]
            kt3 = [t[:].rearrange("p (c n) -> p c n", c=2) for t in d["ktn"]]
            mk = d["mk"]
            fillers = list(fillers)
            fpc = (len(fillers) + NSUB - 1) // NSUB if fillers else 0
            pms = []
            d["pm"] = pms  # filled as the loop runs; read by emit_av0/trav_a
            for mc in range(NSUB):
                stp = ps.tile([P, N], f32, tag="st", bufs=2,
                              name=f"st{b}_{mc}")
                mk3 = mk[:, mc * N:(mc + 1) * N].rearrange(
                    "p (c n) -> p c n", c=2)
                for nh in range(2):
                    nsl = slice(nh * 512, (nh + 1) * 512)
                    nc.tensor.matmul(
                        stp[:, nsl],
                        kt3[mc // 4][:, :, (mc % 4) * P:(mc % 4 + 1) * P],
                        qt3[nh][:], start=True, stop=False, perf_mode=DR)
                    nc.tensor.matmul(
                        stp[:, nsl],
                        idp[nh][:].rearrange("p (c m) -> p c m", c=2),
                        mk3[:], start=False, stop=True, perf_mode=DR)
                if mc % 2 == 0:
                    pm = sb.tile([P, 2 * N], e4, tag="pm", bufs=3 * NPAIR,
                                 name=f"pm{b}_{mc // 2}")
                    pms.append(pm)
                nc.scalar.activation(
                    pms[-1][:, (mc % 2) * N:(mc % 2 + 1) * N], stp[:], AF.Exp)
                # AV0 for pair j is emitted two chunks late (at mc=2j+3) so
                # its wait on exp(2j+1) never sits ahead of the next S^T in
                # the in-order PE queue; the last pair lands in trav_a.
                if mc % 2 == 1 and mc >= 3:
                    j = mc // 2 - 1
                    if j == 0:
                        # lazy alloc keeps the "av" ring ordered with the
                        # previous batch's deferred av1 tiles
                        d["av0"] = [ps.tile([P, 512], f32, tag="av", bufs=2,
                                            name=f"av0_{b}_{hc}")
                                    for hc in range(2)]
                    emit_av0(b, j)
                for _ in range(fpc):
                    if fillers:
                        fillers.pop(0)()
            while fillers:
                fillers.pop(0)()

        def emit_av0(b, j):
            d = st[b]
            pm3 = d["pm"][j][:].rearrange("p (c n) -> p c n", c=2)
            v3 = d["v"][j][:].rearrange("p (c h) -> p c h", c=2)
            for hc in range(2):
                nc.tensor.matmul(
                    d["av0"][hc][:], v3[:, :, hc * P:(hc + 1) * P],
                    pm3[:, :, 0:512], start=(j == 0),
                    stop=(j == NPAIR - 1), perf_mode=DR)

        def trav_y_pieces(b):
            """Closures for the post-S work of batch b: deferred AV n-half 1,
            O^T copies, per-n-chunk d+Y+epilogue, chunked output DMAs.
            Run as fillers inside s(b+1) so nothing blocks its exp chain."""
            ones3 = ones2[:].rearrange("p (c o) -> p c o", c=2)

            def trav_a():
                d = st[b]
                emit_av0(b, NPAIR - 1)  # deferred last pair
                d["ot"] = [sb.tile([P, N], f32r, tag="ot", bufs=4,
                                   name=f"ot{b}_{hc}") for hc in range(2)]
                if b == bp - 1:
                    # tail: the st-ring banks are free once the last exp has
                    # read them — av1 there skips the av-ring rotation
                    av1 = [ps.tile([P, N], f32, tag="st", bufs=2,
                                   name=f"av1_{b}_{hc}")[:, 0:512]
                           for hc in range(2)]
                else:
                    av1 = [ps.tile([P, 512], f32, tag="av", bufs=2,
                                   name=f"av1_{b}_{hc}")[:]
                           for hc in range(2)]
                d["av1"] = av1
                nc.scalar.copy(d["ot"][0][:, 0:512], d["av0"][0][:])
                nc.vector.tensor_copy(d["ot"][1][:, 0:512], d["av0"][1][:])
                for j in range(NPAIR):
                    pm3 = d["pm"][j][:].rearrange("p (c n) -> p c n", c=2)
                    v3 = d["v"][j][:].rearrange("p (c h) -> p c h", c=2)
                    for hc in range(2):
                        nc.tensor.matmul(
                            av1[hc], v3[:, :, hc * P:(hc + 1) * P],
                            pm3[:, :, 512:1024], start=(j == 0),
                            stop=(j == NPAIR - 1), perf_mode=DR)

            def trav_b():
                d = st[b]
                nc.scalar.copy(d["ot"][0][:, 512:1024], d["av1"][0])
                nc.vector.tensor_copy(d["ot"][1][:, 512:1024], d["av1"][1])

            def y_pre():
                st[b]["ybig"] = sb.tile([P, NSUB * DOUT], f32, tag="y",
                                        bufs=2, name=f"y{b}")

            def emit_y(ns):
                d = st[b]
                nsl = slice(ns * P, (ns + 1) * P)
                yp = ps.tile([P, 512], f32, tag="ps", bufs=2,
                             name=f"yp{b}_{ns}")
                for j in range(NPAIR):
                    pm3 = d["pm"][j][:].rearrange("p (c n) -> p c n", c=2)
                    nc.tensor.matmul(
                        yp[:, DOUT:DOUT + 1], pm3[:, :, nsl], ones3[:],
                        start=(j == 0), stop=(j == NPAIR - 1), perf_mode=DR)
                for hc in range(2):
                    nc.tensor.matmul(
                        yp[:, 0:DOUT], d["ot"][hc][:, nsl],
                        wo_sb[:, hc * DOUT:(hc + 1) * DOUT],
                        start=(hc == 0), stop=(hc == 1))
                iv = sb.tile([P, 1], f32, tag="iv", bufs=4,
                             name=f"iv{b}_{ns}")
                nc.vector.reciprocal(iv[:], yp[:, DOUT:DOUT + 1])
                if ns % 2 and b == bp - 1:
                    nc.scalar.activation(
                        ybig_of(b)[:, ns * DOUT:(ns + 1) * DOUT],
                        yp[:, 0:DOUT], AF.Relu, scale=iv[:])
                else:
                    nc.vector.tensor_scalar(
                        out=ybig_of(b)[:, ns * DOUT:(ns + 1) * DOUT],
                        in0=yp[:, 0:DOUT],
                        scalar1=iv[:], scalar2=0.0, op0=ALU.mult, op1=ALU.max)

            def ybig_of(b):
                return st[b]["ybig"]

            def emit_out(q):
                # quarter-batch output DMA right after its data is ready so
                # the SP queue is never held on a long semaphore wait
                csl = slice(q * 2 * DOUT, (q + 1) * 2 * DOUT)
                nc.sync.dma_start(out_d[b, :, csl], ybig_of(b)[:, csl])
                if q == 3:
                    del st[b]

            pieces = [trav_a, trav_b, y_pre]
            for ns in range(NSUB):
                pieces.append(lambda ns=ns: emit_y(ns))
                if ns % 2 == 1:
                    pieces.append(lambda q=ns // 2: emit_out(q))
            return pieces

        # ---- interleaved emission ----
        # s(b) drains fillers between m-chunks: the previous batch's
        # trav/Y/output pieces merged round-robin with batch b+1's input
        # DMAs and QKV so the epilogue engines never burst.
        v0_pieces = qkv0_pieces()
        prev = []
        for b in range(bp):
            nxt = list(v0_pieces) if b == 0 else []
            v0_pieces = []
            if b + 1 < bp:
                nxt.append(lambda bb=b + 1: dma_in(bb))
                nxt.extend(qkv_pieces(b + 1))
            a, c = list(prev), list(nxt)
            fillers = []
            while a or c:
                if a:
                    fillers.append(a.pop(0))
                for _ in range(2):
                    if c:
                        fillers.append(c.pop(0))
            s_phase(b, fillers)
            prev = trav_y_pieces(b)
        for f in prev:
            f()

    nc.compile()
    return nc


def _get_nc(bp=BP, zero_bias=True):
    key = (bp, zero_bias)
    if key not in _nc_cache:
        _nc_cache[key] = _build_nc(bp, zero_bias)
    return _nc_cache[key]


def _pack_inputs(x, mask, Wv, Wk, Wq, Wo, bq, bk):
    import ml_dtypes

    e4 = ml_dtypes.float8_e4m3
    bf = ml_dtypes.bfloat16
    x = np.asarray(x, np.float32)
    b = x.shape[0]
    # x^T packed [b, p, c*N+n]
    xT = x.transpose(0, 2, 1).reshape(b, 2, P, N).transpose(0, 2, 1, 3)
    xT = np.ascontiguousarray(xT.reshape(b, P, 2 * N)).astype(bf)
    # (mask^T - 1) * 32 packed [b, p, mc*N+n]
    # additive mask {unmasked: -5, masked: -36}: a -5 softmax shift
    # keeps exp(S-5) inside float8-e4m3 range; -36 flushes to exact 0
    mk = np.asarray(mask, np.float32).transpose(0, 2, 1) * 31.0 - 36.0
    mk = mk.reshape(b, NSUB, P, N).transpose(0, 2, 1, 3)
    mk = np.ascontiguousarray(mk.reshape(b, P, NSUB * N)).astype(e4)

    def packw(w, dt, scale):
        w = np.asarray(w, np.float32) * scale
        return np.ascontiguousarray(
            w.reshape(2, P, -1).transpose(1, 0, 2).reshape(P, -1)).astype(dt)

    return {
        "xT": xT, "xT8": xT.astype(e4), "mask": mk,
        "Wq": packw(Wq, bf, 1.0),
        "Wk": packw(Wk, bf, 1.0),
        "Wv": packw(Wv, e4, 32.0),
        "Wo": packw(Wo, np.float32, 1.0),
        "bq": np.asarray(bq, np.float32).reshape(H, 1).copy(),
        "bk": np.asarray(bk, np.float32).reshape(H, 1).copy(),
    }


def kernel(x, mask, Wv, bv, Wk, bk, Wq, bq, Wo, bo):
    global last_results
    from concourse.bass_utils import run_bass_kernel_spmd

    if np.any(np.asarray(bv, np.float32)) or np.any(np.asarray(bo, np.float32)):
        raise NotImplementedError("nonzero bv/bo not supported")
    zero_bias = not (np.any(np.asarray(bq, np.float32))
                     or np.any(np.asarray(bk, np.float32)))

    w = _pack_inputs(x, mask, Wv, Wk, Wq, Wo, bq, bk)
    nc = _get_nc(BP, zero_bias)
    in_maps = []
    for c in range(NCORES):
        sl = slice(c * BP, (c + 1) * BP)
        m = {"xT": np.ascontiguousarray(w["xT"][sl]),
             "xT8": np.ascontiguousarray(w["xT8"][sl]),
             "mask": np.ascontiguousarray(w["mask"][sl])}
        for k in ("Wq", "Wk", "Wv", "Wo", "bq", "bk"):
            m[k] = w[k]
        in_maps.append(m)

    trace = bool(int(os.environ.get("BASS_KERNEL_TRACE", "0")))
    try:
        res = run_bass_kernel_spmd(
            nc, in_maps, core_ids=list(range(NCORES)), trace=trace
        )
    except Exception:
        if not trace:
            raise
        res = run_bass_kernel_spmd(nc, in_maps, core_ids=list(range(NCORES)))
    last_results = res
    # out comes back packed [bp, p, ns*DOUT+o] bf16 -> [B, N, DOUT] f32
    outs = []
    for r in res.results:
        y = np.asarray(r["out"], np.float32).reshape(BP, P, NSUB, DOUT)
        outs.append(y.transpose(0, 2, 1, 3).reshape(BP, N, DOUT))
    return np.ascontiguousarray(np.concatenate(outs, axis=0))


if __name__ == "__main__":
    nc = _get_nc(1)
    print("built ok:", nc)


# revision 47
# speedup vs baseline: 1.0170x; 1.0107x over previous
"""Trainium2 Bass kernel for nn_AttModel (masked GNN attention).

Reference computation (per batch b of 32, N=1024, D=H=O=256):
    v = relu(x @ Wv); q = relu(x @ Wq); k = relu(x @ Wk)   (biases are zero)
    S = q @ k^T
    att = softmax(S * mask - 9e15 * (1 - mask), axis=-1)
    out = relu((att @ v) @ Wo)

Strategy: pure data parallelism over batch — 8 NeuronCores, 4 batches
each, weights replicated, no collectives.  Per batch, everything is
built around fp8 DoubleRow matmuls (0.5 cyc/row, K=256 packed per
instruction) and a transposed-S dataflow that needs no PE transposes:

  - Host packs x^T and the QKV weights as bf16 (fp8 x/W cost ~1.2e-2
    of output error, the dominant term) and the additive mask as
    float8-e4m3 (mask*31-36: a -5 softmax shift keeping exp(S-5) in
    e4m3 range, -36 masking that exp+e4m3 flushes to exact 0); Wo is
    f32r.  DRAM layouts are partition-major full-rate 2D transfers.
  - Q^T/K^T/V projections are plain bf16 matmuls; q/k/v quantize to
    e4m3 in the relu epilogues (DVE/ACT) for the fp8-DR stages.
  - S^T[m, n] = K Q^T accumulated in PSUM; the additive mask rides into
    the same accumulation group as a second fp8-DR matmul against a
    packed identity.
  - One ACT exp per m-chunk ([128,1024] PSUM -> SBUF e4m3 pm).  ACT is
    the bottleneck engine (~8.3us/batch); everything else is scheduled
    to keep it fed.
  - AV: O^T[h, n] accumulates pm-pairs straight from SBUF (fp8-DR);
    n-half 1 is deferred past the S loop to stay within 8 PSUM banks
    (st 2x2 + av 2x1 + qkv/y 2x1).
  - Softmax denominators d[n] come from Nf=1 fp8-DR matmuls (pm
    stationary, ones moving) into a spare PSUM column of the Y tile;
    Y = O^T.T @ Wo runs in f32r (fp8 there would amplify cancellation
    noise); DVE does reciprocal + (mult,max); the host unpacks the
    partition-major f32 output.
  - Emission is interleaved at m-chunk granularity: batch b+1's QKV and
    batch b-1's Y ride as fillers inside batch b's S loop so the PE,
    DVE, Pool and DMA queues all stay busy without blocking the
    exp chain.
"""

import os

import numpy as np

B, N, DIN, H, DOUT = 32, 1024, 256, 256, 256
NCORES = 8
BP = B // NCORES  # batches per core
P = 128
NSUB = N // P   # 8 m-chunks of 128
NPAIR = NSUB // 2  # 4 m-pairs (K=256 per DR matmul)

_nc_cache = {}
last_results = None  # BassKernelResults of the most recent run (for test.py)


def _build_nc(bp=BP, zero_bias=True):
    import concourse.mybir as mybir
    import concourse.tile as tile
    from concourse import bacc
    from concourse.masks import make_identity
    from contextlib import ExitStack

    f32 = mybir.dt.float32
    f32r = mybir.dt.float32r
    bf16 = mybir.dt.bfloat16
    e4 = mybir.dt.float8e4
    e5 = mybir.dt.float8e5
    AF = mybir.ActivationFunctionType
    ALU = mybir.AluOpType
    DR = mybir.MatmulPerfMode.DoubleRow

    nc = bacc.Bacc("TRN2", target_bir_lowering=False)

    # x^T packed [bp, p, c*N+n]: value x[b, n, c*128+p]
    xT_d = nc.declare_dram_parameter("xT", [bp, P, 2 * N], bf16,
                                    isOutput=False)
    xT8_d = nc.declare_dram_parameter("xT8", [bp, P, 2 * N], e4,
                                      isOutput=False)
    # additive transposed mask [bp, p, mc*N+n]: (mask[b, n, mc*128+p]-1)*32
    mask_d = nc.declare_dram_parameter("mask", [bp, P, NSUB * N], e4,
                                       isOutput=False)
    wq_d = nc.declare_dram_parameter("Wq", [P, 2 * H], bf16, isOutput=False)
    wk_d = nc.declare_dram_parameter("Wk", [P, 2 * H], bf16, isOutput=False)
    wv_d = nc.declare_dram_parameter("Wv", [P, 2 * H], e4, isOutput=False)
    wo_d = nc.declare_dram_parameter("Wo", [P, 2 * DOUT], f32r,
                                     isOutput=False)
    bq_d = nc.declare_dram_parameter("bq", [H, 1], f32, isOutput=False)
    bk_d = nc.declare_dram_parameter("bk", [H, 1], f32, isOutput=False)
    # y packed [bp, p, ns*DOUT+o]: value y[b, ns*128+p, o] (host unpacks)
    out_d = nc.declare_dram_parameter("out", [bp, P, NSUB * DOUT], f32,
                                      isOutput=True)

    inv = 1.0
    inv32 = 1.0 / 32.0

    with tile.TileContext(nc) as tc, ExitStack() as ctx:
        const = ctx.enter_context(tc.tile_pool(name="const", bufs=1))
        sb = ctx.enter_context(tc.tile_pool(name="sb", bufs=1))
        ps = ctx.enter_context(tc.tile_pool(name="ps", bufs=1, space="PSUM"))

        st = {}

        # ---- batch-0 x first, then weights, then batch-0 mask: the sim's
        # DMA engines drain in order, so the first QT must not sit behind
        # the 1MB mask transfer ----
        def dma_x(b, defer8=False):
            d = st.setdefault(b, {})
            xt = sb.tile([P, 2 * N], bf16, tag="xt", bufs=3, name=f"xt{b}")
            nc.sync.dma_start(xt[:], xT_d[b])
            d["xt"] = xt
            if not defer8:
                dma_x8(b)

        def dma_x8(b):
            d = st[b]
            xt8 = sb.tile([P, 2 * N], e4, tag="xt8", bufs=3, name=f"xt8{b}")
            nc.sync.dma_start(xt8[:], xT8_d[b])
            d["xt8"] = xt8

        def dma_mask(b, split=1):
            d = st.setdefault(b, {})
            mk = sb.tile([P, NSUB * N], e4, tag="mk", bufs=3, name=f"mk{b}")
            step = NSUB * N // split
            for s in range(split):
                nc.sync.dma_start(mk[:, s * step:(s + 1) * step],
                                  mask_d[b, :, s * step:(s + 1) * step])
            d["mk"] = mk

        def dma_in(b):
            dma_x(b)
            dma_mask(b)

        # batch-0 path: x, first mask quarter and wq/wk land first so the
        # first S^T chunk starts ~4us in; the serial DMA engine otherwise
        # parks the whole 1MB mask in front of the weights
        dma_x(0, defer8=True)
        mk0 = sb.tile([P, NSUB * N], e4, tag="mk", bufs=3, name="mk0")
        st.setdefault(0, {})["mk"] = mk0
        nc.sync.dma_start(mk0[:, :2 * N], mask_d[0, :, :2 * N])
        wq_sb = const.tile([P, 2 * H], bf16, tag="wq", name="wq_sb")
        nc.sync.dma_start(wq_sb[:], wq_d[:])
        wk_sb = const.tile([P, 2 * H], bf16, tag="wk", name="wk_sb")
        nc.sync.dma_start(wk_sb[:], wk_d[:])
        nc.sync.dma_start(mk0[:, 2 * N:4 * N], mask_d[0, :, 2 * N:4 * N])
        dma_x8(0)
        wv_sb = const.tile([P, 2 * H], e4, tag="wv", name="wv_sb")
        nc.sync.dma_start(wv_sb[:], wv_d[:])
        if not zero_bias:
            bq_sb = const.tile([P, 2], f32, tag="bq", name="bq_sb")
            nc.gpsimd.dma_start(
                bq_sb[:].rearrange("p (c o) -> p c o", c=2),
                bq_d.rearrange("(c p) o -> p c o", c=2))
            bk_sb = const.tile([P, 2], f32, tag="bk", name="bk_sb")
            nc.gpsimd.dma_start(
                bk_sb[:].rearrange("p (c o) -> p c o", c=2),
                bk_d.rearrange("(c p) o -> p c o", c=2))

        # identity packs for the mask-add matmul: idp[nh] has I at k-tile nh
        idp = []
        for nh in range(2):
            t = const.tile([P, 2 * P], e4, tag=f"idp{nh}", name=f"idp{nh}")
            nc.gpsimd.memset(t[:], 0.0)
            make_identity(nc, t[:, nh * P:(nh + 1) * P], nomemset=True)
            idp.append(t)
        ones2 = const.tile([P, 2], e4, tag="ones2", name="ones2")
        nc.gpsimd.memset(ones2[:], 1.0)

        wo_sb = const.tile([P, 2 * DOUT], f32r, tag="wo", name="wo_sb")
        nc.sync.dma_start(wo_sb[:], wo_d[:])
        nc.sync.dma_start(mk0[:, 4 * N:], mask_d[0, :, 4 * N:])

        def qkv0_pieces():
            """Batch-0 QKV with QT/V on the idle st-ring (4 banks) and KT on
            the ps-ring, so the startup is not paced by a single 2-deep
            ring; steady-state batches overlap a full S phase instead."""
            d = st[0]
            alloc_qk(0)
            d["v"] = []
            # nh=0 halves first: the first S^T chunk needs only qtA/ktA
            for nh in range(2):
                for hc in range(2):
                    pq = ps.tile([P, N], f32, tag="st", bufs=2,
                                 name=f"pq0_{hc}_{nh}")
                    for dc in range(2):
                        nc.tensor.matmul(
                            pq[:, 0:512],
                            wq_sb[:, dc * H + hc * P:dc * H + (hc + 1) * P],
                            d["xt"][:, dc * N + nh * 512:
                                    dc * N + (nh + 1) * 512],
                            start=(dc == 0), stop=(dc == 1))
                    nc.vector.tensor_scalar(
                        out=d["qtn"][nh][:, hc * 512:(hc + 1) * 512],
                        in0=pq[:, 0:512], scalar1=inv,
                        scalar2=0.0, op0=ALU.mult, op1=ALU.max)
                    pk = ps.tile([P, 512], f32, tag="ps", bufs=2,
                                 name=f"pk0_{hc}_{nh}")
                    for dc in range(2):
                        nc.tensor.matmul(
                            pk[:],
                            wk_sb[:, dc * H + hc * P:dc * H + (hc + 1) * P],
                            d["xt"][:, dc * N + nh * 512:
                                    dc * N + (nh + 1) * 512],
                            start=(dc == 0), stop=(dc == 1))
                    if hc == 0:
                        nc.scalar.activation(
                            d["ktn"][nh][:, hc * 512:(hc + 1) * 512], pk[:],
                            AF.Relu, scale=inv)
                    else:
                        nc.vector.tensor_scalar(
                            out=d["ktn"][nh][:, hc * 512:(hc + 1) * 512],
                            in0=pk[:], scalar1=inv, scalar2=0.0,
                            op0=ALU.mult, op1=ALU.max)
            def emit_v0(jj):
                pv = ps.tile([P, 512], f32, tag="ps", bufs=2,
                             name=f"pv0_{jj}")
                xt83 = d["xt8"][:].rearrange("p (c n) -> p c n", c=2)
                wv3 = wv_sb[:].rearrange("p (c h) -> p c h", c=2)
                for c2 in range(2):
                    mc = 2 * jj + c2
                    nc.tensor.matmul(
                        pv[:, c2 * H:(c2 + 1) * H],
                        xt83[:, :, mc * P:(mc + 1) * P], wv3[:],
                        start=True, stop=True, perf_mode=DR)
                v = sb.tile([P, 2 * H], e4, tag="v", bufs=2 * NPAIR,
                            name=f"v0_{jj}")
                if jj % 2:
                    nc.vector.tensor_scalar(
                        out=v[:], in0=pv[:], scalar1=inv32, scalar2=0.0,
                        op0=ALU.mult, op1=ALU.max)
                else:
                    nc.scalar.activation(v[:], pv[:], AF.Relu, scale=inv32)
                d["v"].append(v)

            # V is first needed at mc2 of s(0): ride the fillers instead of
            # blocking the first S^T chunks in the in-order PE queue
            return [lambda jj=jj: emit_v0(jj) for jj in range(NPAIR)]

        def alloc_qk(b):
            # q and k live as per-n-half / per-m-half tiles ([p, (hc n)]) so
            # a consumer's (coarse, tile-granular) dependency covers only
            # the half it actually reads
            d = st.setdefault(b, {})
            d["qtn"] = [sb.tile([P, N], e4, tag=f"qt{nh}", bufs=2,
                                name=f"qt{b}_{nh}") for nh in range(2)]
            d["ktn"] = [sb.tile([P, N], e4, tag=f"kt{nh}", bufs=2,
                                name=f"kt{b}_{nh}") for nh in range(2)]

        def qkv_pieces(b):
            """Returns emission closures: Q^T/K^T/V projections for batch b.
            State is resolved lazily so the dma_in(b) filler can run first."""

            def prelude():
                alloc_qk(b)
                st[b]["v"] = []

            def emit_qk(use_q, hc, nh, on_dve):
                d = st[b]
                w_sb = wq_sb if use_q else wk_sb
                dst = (d["qtn"] if use_q else d["ktn"])[nh]
                pq = ps.tile([P, 512], f32, tag="ps", bufs=2,
                             name=f"pqk{b}_{hc}_{nh}")
                for dc in range(2):
                    nc.tensor.matmul(
                        pq[:],
                        w_sb[:, dc * H + hc * P:dc * H + (hc + 1) * P],
                        d["xt"][:, dc * N + nh * 512:dc * N + (nh + 1) * 512],
                        start=(dc == 0), stop=(dc == 1))
                qsl = slice(hc * 512, (hc + 1) * 512)
                if not zero_bias:
                    bias = (bq_sb if use_q else bk_sb)[:, hc:hc + 1]
                    nc.scalar.activation(dst[:, qsl], pq[:], AF.Relu,
                                         bias=bias, scale=inv)
                elif on_dve:
                    nc.vector.tensor_scalar(
                        out=dst[:, qsl], in0=pq[:], scalar1=inv,
                        scalar2=0.0, op0=ALU.mult, op1=ALU.max)
                else:
                    nc.scalar.activation(dst[:, qsl], pq[:], AF.Relu,
                                         scale=inv)

            def emit_v(j):
                d = st[b]
                pv = ps.tile([P, 512], f32, tag="ps", bufs=2,
                             name=f"pv{b}_{j}")
                xt83 = d["xt8"][:].rearrange("p (c n) -> p c n", c=2)
                wv3 = wv_sb[:].rearrange("p (c h) -> p c h", c=2)
                for c2 in range(2):
                    mc = 2 * j + c2
                    nc.tensor.matmul(
                        pv[:, c2 * H:(c2 + 1) * H],
                        xt83[:, :, mc * P:(mc + 1) * P], wv3[:],
                        start=True, stop=True, perf_mode=DR)
                v = sb.tile([P, 2 * H], e4, tag="v", bufs=2 * NPAIR,
                            name=f"v{b}_{j}")
                nc.vector.tensor_scalar(
                    out=v[:], in0=pv[:], scalar1=inv32, scalar2=0.0,
                    op0=ALU.mult, op1=ALU.max)
                d["v"].append(v)

            pieces = [prelude]
            for hc in range(2):
                for nh in range(2):
                    # alternate DVE (q) / Pool (k) so neither queue bursts
                    pieces.append(
                        lambda hc=hc, nh=nh: emit_qk(True, hc, nh, True))
                    # kt epilogues split ACT (nh0) / DVE (nh1) for balance
                    pieces.append(
                        lambda hc=hc, nh=nh: emit_qk(False, hc, nh, nh == 1))
            for j in range(NPAIR):
                # one v epilogue per batch goes to DVE to balance Pool
                pieces.append(lambda j=j: emit_v(j))
            return pieces

        def s_phase(b, fillers=()):
            """S^T + mask (PE) -> exp (ACT) -> AV n-half 0 (PE), with
            filler closures from other batches drained between chunks."""
            d = st[b]
            qt3 = [t[:].rearrange("p (c n) -> p c n", c=2) for t in d["qtn"]]
            kt3 = [t[:].rearrange("p (c n) -> p c n", c=2) for t in d["ktn"]]
            mk = d["mk"]
            fillers = list(fillers)
            fpc = (len(fillers) + NSUB - 1) // NSUB if fillers else 0
            pms = []
            d["pm"] = pms  # filled as the loop runs; read by emit_av0/trav_a
            for mc in range(NSUB):
                stp = ps.tile([P, N], f32, tag="st", bufs=2,
                              name=f"st{b}_{mc}")
                mk3 = mk[:, mc * N:(mc + 1) * N].rearrange(
                    "p (c n) -> p c n", c=2)
                for nh in range(2):
                    nsl = slice(nh * 512, (nh + 1) * 512)
                    nc.tensor.matmul(
                        stp[:, nsl],
                        kt3[mc // 4][:, :, (mc % 4) * P:(mc % 4 + 1) * P],
                        qt3[nh][:], start=True, stop=False, perf_mode=DR)
                    nc.tensor.matmul(
                        stp[:, nsl],
                        idp[nh][:].rearrange("p (c m) -> p c m", c=2),
                        mk3[:], start=False, stop=True, perf_mode=DR)
                if mc % 2 == 0:
                    pm = sb.tile([P, 2 * N], e4, tag="pm", bufs=3 * NPAIR,
                                 name=f"pm{b}_{mc // 2}")
                    pms.append(pm)
                nc.scalar.activation(
                    pms[-1][:, (mc % 2) * N:(mc % 2 + 1) * N], stp[:], AF.Exp)
                # AV0 for pair j is emitted two chunks late (at mc=2j+3) so
                # its wait on exp(2j+1) never sits ahead of the next S^T in
                # the in-order PE queue; the last pair lands in trav_a.
                if mc % 2 == 1 and mc >= 3:
                    j = mc // 2 - 1
                    if j == 0:
                        # lazy alloc keeps the "av" ring ordered with the
                        # previous batch's deferred av1 tiles
                        d["av0"] = [ps.tile([P, 512], f32, tag="av", bufs=2,
                                            name=f"av0_{b}_{hc}")
                                    for hc in range(2)]
                    emit_av0(b, j)
                for _ in range(fpc):
                    if fillers:
                        fillers.pop(0)()
            while fillers:
                fillers.pop(0)()

        def emit_av0(b, j):
            d = st[b]
            pm3 = d["pm"][j][:].rearrange("p (c n) -> p c n", c=2)
            v3 = d["v"][j][:].rearrange("p (c h) -> p c h", c=2)
            for hc in range(2):
                nc.tensor.matmul(
                    d["av0"][hc][:], v3[:, :, hc * P:(hc + 1) * P],
                    pm3[:, :, 0:512], start=(j == 0),
                    stop=(j == NPAIR - 1), perf_mode=DR)

        def trav_y_pieces(b):
            """Closures for the post-S work of batch b: deferred AV n-half 1,
            O^T copies, per-n-chunk d+Y+epilogue, chunked output DMAs.
            Run as fillers inside s(b+1) so nothing blocks its exp chain."""
            ones3 = ones2[:].rearrange("p (c o) -> p c o", c=2)

            def trav_a():
                d = st[b]
                emit_av0(b, NPAIR - 1)  # deferred last pair
                d["ot"] = [sb.tile([P, N], f32r, tag="ot", bufs=4,
                                   name=f"ot{b}_{hc}") for hc in range(2)]
                if b == bp - 1:
                    # tail: the st-ring banks are free once the last exp has
                    # read them — av1 there skips the av-ring rotation
                    av1 = [ps.tile([P, N], f32, tag="st", bufs=2,
                                   name=f"av1_{b}_{hc}")[:, 0:512]
                           for hc in range(2)]
                else:
                    av1 = [ps.tile([P, 512], f32, tag="av", bufs=2,
                                   name=f"av1_{b}_{hc}")[:]
                           for hc in range(2)]
                d["av1"] = av1
                nc.scalar.copy(d["ot"][0][:, 0:512], d["av0"][0][:])
                nc.vector.tensor_copy(d["ot"][1][:, 0:512], d["av0"][1][:])
                for j in range(NPAIR):
                    pm3 = d["pm"][j][:].rearrange("p (c n) -> p c n", c=2)
                    v3 = d["v"][j][:].rearrange("p (c h) -> p c h", c=2)
                    for hc in range(2):
                        nc.tensor.matmul(
                            av1[hc], v3[:, :, hc * P:(hc + 1) * P],
                            pm3[:, :, 512:1024], start=(j == 0),
                            stop=(j == NPAIR - 1), perf_mode=DR)

            def trav_b():
                d = st[b]
                nc.scalar.copy(d["ot"][0][:, 512:1024], d["av1"][0])
                nc.vector.tensor_copy(d["ot"][1][:, 512:1024], d["av1"][1])

            def y_pre():
                st[b]["ybig"] = sb.tile([P, NSUB * DOUT], f32, tag="y",
                                        bufs=2, name=f"y{b}")

            def emit_y(ns):
                d = st[b]
                nsl = slice(ns * P, (ns + 1) * P)
                yp = ps.tile([P, 512], f32, tag="ps", bufs=2,
                             name=f"yp{b}_{ns}")
                for j in range(NPAIR):
                    pm3 = d["pm"][j][:].rearrange("p (c n) -> p c n", c=2)
                    nc.tensor.matmul(
                        yp[:, DOUT:DOUT + 1], pm3[:, :, nsl], ones3[:],
                        start=(j == 0), stop=(j == NPAIR - 1), perf_mode=DR)
                for hc in range(2):
                    nc.tensor.matmul(
                        yp[:, 0:DOUT], d["ot"][hc][:, nsl],
                        wo_sb[:, hc * DOUT:(hc + 1) * DOUT],
                        start=(hc == 0), stop=(hc == 1))
                iv = sb.tile([P, 1], f32, tag="iv", bufs=4,
                             name=f"iv{b}_{ns}")
                nc.vector.reciprocal(iv[:], yp[:, DOUT:DOUT + 1])
                if ns % 2 and b == bp - 1:
                    nc.scalar.activation(
                        ybig_of(b)[:, ns * DOUT:(ns + 1) * DOUT],
                        yp[:, 0:DOUT], AF.Relu, scale=iv[:])
                else:
                    nc.vector.tensor_scalar(
                        out=ybig_of(b)[:, ns * DOUT:(ns + 1) * DOUT],
                        in0=yp[:, 0:DOUT],
                        scalar1=iv[:], scalar2=0.0, op0=ALU.mult, op1=ALU.max)

            def ybig_of(b):
                return st[b]["ybig"]

            def emit_out(q):
                # quarter-batch output DMA right after its data is ready so
                # the SP queue is never held on a long semaphore wait
                csl = slice(q * 2 * DOUT, (q + 1) * 2 * DOUT)
                nc.sync.dma_start(out_d[b, :, csl], ybig_of(b)[:, csl])
                if q == 3:
                    del st[b]

            pieces = [trav_a, trav_b, y_pre]
            for ns in range(NSUB):
                pieces.append(lambda ns=ns: emit_y(ns))
                if ns % 2 == 1:
                    pieces.append(lambda q=ns // 2: emit_out(q))
            return pieces

        # ---- interleaved emission ----
        # s(b) drains fillers between m-chunks: the previous batch's
        # trav/Y/output pieces merged round-robin with batch b+1's input
        # DMAs and QKV so the epilogue engines never burst.
        v0_pieces = qkv0_pieces()
        prev = []
        for b in range(bp):
            nxt = list(v0_pieces) if b == 0 else []
            v0_pieces = []
            if b + 1 < bp:
                nxt.append(lambda bb=b + 1: dma_in(bb))
                nxt.extend(qkv_pieces(b + 1))
            a, c = list(prev), list(nxt)
            fillers = []
            while a or c:
                if a:
                    fillers.append(a.pop(0))
                for _ in range(2):
                    if c:
                        fillers.append(c.pop(0))
            s_phase(b, fillers)
            prev = trav_y_pieces(b)
        for f in prev:
            f()

    nc.compile()
    return nc


def _get_nc(bp=BP, zero_bias=True):
    key = (bp, zero_bias)
    if key not in _nc_cache:
        _nc_cache[key] = _build_nc(bp, zero_bias)
    return _nc_cache[key]


def _pack_inputs(x, mask, Wv, Wk, Wq, Wo, bq, bk):
    import ml_dtypes

    e4 = ml_dtypes.float8_e4m3
    bf = ml_dtypes.bfloat16
    x = np.asarray(x, np.float32)
    b = x.shape[0]
    # x^T packed [b, p, c*N+n]
    xT = x.transpose(0, 2, 1).reshape(b, 2, P, N).transpose(0, 2, 1, 3)
    xT = np.ascontiguousarray(xT.reshape(b, P, 2 * N)).astype(bf)
    # (mask^T - 1) * 32 packed [b, p, mc*N+n]
    # additive mask {unmasked: -5, masked: -36}: a -5 softmax shift
    # keeps exp(S-5) inside float8-e4m3 range; -36 flushes to exact 0
    mk = np.asarray(mask, np.float32).transpose(0, 2, 1) * 31.0 - 36.0
    mk = mk.reshape(b, NSUB, P, N).transpose(0, 2, 1, 3)
    mk = np.ascontiguousarray(mk.reshape(b, P, NSUB * N)).astype(e4)

    def packw(w, dt, scale):
        w = np.asarray(w, np.float32) * scale
        return np.ascontiguousarray(
            w.reshape(2, P, -1).transpose(1, 0, 2).reshape(P, -1)).astype(dt)

    return {
        "xT": xT, "xT8": xT.astype(e4), "mask": mk,
        "Wq": packw(Wq, bf, 1.0),
        "Wk": packw(Wk, bf, 1.0),
        "Wv": packw(Wv, e4, 32.0),
        "Wo": packw(Wo, np.float32, 1.0),
        "bq": np.asarray(bq, np.float32).reshape(H, 1).copy(),
        "bk": np.asarray(bk, np.float32).reshape(H, 1).copy(),
    }


def kernel(x, mask, Wv, bv, Wk, bk, Wq, bq, Wo, bo):
    global last_results
    from concourse.bass_utils import run_bass_kernel_spmd

    if np.any(np.asarray(bv, np.float32)) or np.any(np.asarray(bo, np.float32)):
        raise NotImplementedError("nonzero bv/bo not supported")
    zero_bias = not (np.any(np.asarray(bq, np.float32))
                     or np.any(np.asarray(bk, np.float32)))

    w = _pack_inputs(x, mask, Wv, Wk, Wq, Wo, bq, bk)
    nc = _get_nc(BP, zero_bias)
    in_maps = []
    for c in range(NCORES):
        sl = slice(c * BP, (c + 1) * BP)
        m = {"xT": np.ascontiguousarray(w["xT"][sl]),
             "xT8": np.ascontiguousarray(w["xT8"][sl]),
             "mask": np.ascontiguousarray(w["mask"][sl])}
        for k in ("Wq", "Wk", "Wv", "Wo", "bq", "bk"):
            m[k] = w[k]
        in_maps.append(m)

    trace = bool(int(os.environ.get("BASS_KERNEL_TRACE", "0")))
    try:
        res = run_bass_kernel_spmd(
            nc, in_maps, core_ids=list(range(NCORES)), trace=trace
        )
    except Exception:
        if not trace:
            raise
        res = run_bass_kernel_spmd(nc, in_maps, core_ids=list(range(NCORES)))
    last_results = res
    # out comes back packed [bp, p, ns*DOUT+o] bf16 -> [B, N, DOUT] f32
    outs = []
    for r in res.results:
        y = np.asarray(r["out"], np.float32).reshape(BP, P, NSUB, DOUT)
        outs.append(y.transpose(0, 2, 1, 3).reshape(BP, N, DOUT))
    return np.ascontiguousarray(np.concatenate(outs, axis=0))


if __name__ == "__main__":
    nc = _get_nc(1)
    print("built ok:", nc)
